# revision 30
# baseline (speedup 1.0000x reference)
"""GAT message-passing kernel for 8 Trainium2 NeuronCores.

Strategy (dst-sharded, per core):
  - Nodes are partitioned across 8 cores by destination id (12500 dst nodes
    per core); every real edge is owned by the core owning its destination.
    Self-loops are computed locally (no gather).
  - Phase 0: each core computes h = x @ W, a_src = <h, att_src>,
    a_dst = <h, att_dst> for its own nodes, packs [h(60) | a_src(4)] rows and
    AllGathers them into a full 100128-row table (each core contributes
    12516 rows: 12500 data + 16 zero rows used as pad targets).
  - Phase 1: per src-chunk k (4 chunks of 25032 table rows so chunk-local
    indices fit int16 for dma_gather), edges are laid out in a CSR slot grid
    [128 dst x D columns] per tile, degree-sorted per chunk so padding is
    small.  One dma_gather per group of tiles fetches packed rows per edge.
    a_dst in chunk-sorted order comes from host-permuted x copies via a tiny
    matmul against wd = W @ att_dst (folded on host), NOT via dma_gather.
    Unnormalized attention w = exp(leaky_relu(a_src + a_dst)) (softmax
    normalization cancels), then per-dst row-sums produce partial
    numerator [60] and denominator [4].
  - Phase 2: the 4 chunk-partial outputs are re-gathered into natural dst
    order, summed; self-loop contributions and pad corrections are applied
    analytically; then out = elu(num/den + bias) @ lin_w + lin_b
    -> log_softmax.
"""
import sys

sys.path.insert(0, "/opt/trn_rl_repo")

import numpy as np

N, E = 100000, 1600000
IN_DIM, HEADS, HID, OUT_DIM = 128, 4, 15, 10
NEG_SLOPE = 0.2
NCORES = 8
NPC = N // NCORES              # 12500 dst nodes per core
T = (NPC + 127) // 128         # 98 tiles
NPCP = 128 * T                 # 12544 padded locals
SH = NPC + 16                  # 12516 shard rows contributed to allgather
CHSZ = N // 4                  # 25000 nodes per chunk (2 cores)
CHROWS = 2 * SH                # 25032 table rows per chunk window
NTAB = NCORES * SH             # 100128 table rows
ROW = 64                       # packed row: h(60) + a_src(4)
F = HEADS * HID                # 60
GROUP_COLS = 96                # max CSR columns per dma_gather


def _wrap_idx(flat):
    """int32 flat index list -> [128, n//16] int16 wrapped layout."""
    n = len(flat)
    assert n % 16 == 0
    w = flat.astype(np.int16).reshape(n // 16, 16).T.copy()
    return np.tile(w, (8, 1))


def _preprocess(src, dst):
    """Build per-core schedules (no self-loops). Returns (D, per_core)."""
    core_of = dst // NPC
    per_core = []
    for c in range(NCORES):
        m = core_of == c
        s_c = src[m].astype(np.int64)
        d_loc = (dst[m] - c * NPC).astype(np.int64)
        chunks = []
        for k in range(4):
            km = (s_c // CHSZ) == k
            sk = s_c[km]
            # chunk-local table row: 2*SH*k + (sk - 25000k) + 16*(second core)
            loc = sk - k * CHSZ
            tloc = np.where(loc < NPC, loc, loc + 16).astype(np.int32)
            dk = d_loc[km].astype(np.int32)
            counts = np.bincount(dk, minlength=NPCP).astype(np.int32)
            order = np.argsort(-counts, kind="stable").astype(np.int32)
            oi = np.empty(NPCP, np.int32)
            oi[order] = np.arange(NPCP, dtype=np.int32)
            D_ck = counts[order[0::128]]
            chunks.append(dict(tloc=tloc, dk=dk, counts=counts, order=order,
                               oi=oi, D_ck=D_ck))
        per_core.append(chunks)
    D = [np.max([per_core[c][k]["D_ck"] for c in range(NCORES)], axis=0)
         .astype(np.int32) for k in range(4)]
    return D, per_core


def _build_core_arrays(D, chunks):
    """Per-core gather index arrays and pad counts."""
    gidx_parts = []
    rgidx_parts = []
    P = np.zeros(NPCP, np.float32)
    for k in range(4):
        d = chunks[k]
        Dk = D[k]
        base = np.concatenate([[0], np.cumsum(128 * Dk.astype(np.int64))])
        NI = int(base[-1])
        # pad target: zero row 12500 of the chunk's first core block
        idx_flat = np.full(NI, NPC, np.int32)
        o = np.argsort(d["dk"], kind="stable")
        dks = d["dk"][o]
        tls = d["tloc"][o]
        starts = np.concatenate([[0], np.cumsum(d["counts"])]).astype(np.int64)
        rank = np.arange(len(dks), dtype=np.int64) - starts[dks]
        oi = d["oi"][dks]
        tau, pp = oi // 128, oi % 128
        linear = base[tau] + rank * 128 + pp
        idx_flat[linear] = tls
        gidx_parts.append(_wrap_idx(idx_flat))
        # combine regather: final slot (p, t) i.e. local = 98p+t, output
        # linear i = t*128 + p  -> sub-out row = p'*T + tau
        loc = (np.arange(NPCP, dtype=np.int32).reshape(T, 128) % 128) * T \
            + np.arange(T, dtype=np.int32)[:, None]
        oiL = d["oi"][loc.reshape(-1)]
        subrow = (oiL % 128) * T + oiL // 128
        rgidx_parts.append(_wrap_idx(subrow.astype(np.int32)))
        P += (Dk[d["oi"] // 128] - d["counts"]).astype(np.float32)
    gidx = np.concatenate(gidx_parts, axis=1)
    rgidx = np.concatenate(rgidx_parts, axis=1)
    # P laid out [128, T] with [p, t] = P[98p + t]
    Parr = P[(np.arange(128)[:, None] * T + np.arange(T)[None, :])]
    return gidx, rgidx, Parr.astype(np.float32)


def _groups_of(Dk):
    """Split tiles into gather groups with <= GROUP_COLS columns."""
    groups = []
    cur = []
    cols = 0
    for tau in range(T):
        dcol = int(Dk[tau])
        if dcol == 0:
            continue
        if cols + dcol > GROUP_COLS and cur:
            groups.append(cur)
            cur = []
            cols = 0
        cur.append(tau)
        cols += dcol
    if cur:
        groups.append(cur)
    return groups


RUN_COLS = 48


def _runs_of(Dk, group):
    """Split a group's consecutive taus into equal-D runs <= RUN_COLS cols.

    Returns [(tau0, ntile, Dv), ...]; D is non-increasing so equal-D tiles
    are consecutive.
    """
    runs = []
    i = 0
    while i < len(group):
        Dv = int(Dk[group[i]])
        j = i
        cols = 0
        while j < len(group) and int(Dk[group[j]]) == Dv \
                and cols + Dv <= RUN_COLS:
            cols += Dv
            j += 1
        if j == i:
            j = i + 1
        runs.append((group[i], j - i, Dv))
        i = j
    return runs


def _build_program(D):
    import concourse.bass as bass
    import concourse.bacc as bacc
    import concourse.tile as tile
    from concourse import mybir
    from concourse.masks import make_identity

    fp32 = mybir.dt.float32
    i16 = mybir.dt.int16
    AL = mybir.AluOpType
    AF = mybir.ActivationFunctionType

    NI = [int((128 * D[k].astype(np.int64)).sum()) for k in range(4)]
    NItot = sum(NI)

    nc = bacc.Bacc("TRN2", target_bir_lowering=False, debug=False,
                   num_devices=NCORES)

    xT = nc.dram_tensor("xT", [128, NPCP], fp32, kind="ExternalInput").ap()
    # x columns permuted into each chunk's degree-sorted slot order
    xs_in = [nc.dram_tensor(f"xs{k}", [128, NPCP], fp32,
                            kind="ExternalInput").ap() for k in range(4)]
    # w68 = [W(60) | W@att_src(4) | W@att_dst(4)] folded on host
    w_in = nc.dram_tensor("w_in", [128, F + 2 * HEADS], fp32,
                          kind="ExternalInput").ap()
    wd_in = nc.dram_tensor("wd_in", [128, HEADS], fp32, kind="ExternalInput").ap()
    bias_in = nc.dram_tensor("bias_in", [128, F], fp32, kind="ExternalInput").ap()
    linw_in = nc.dram_tensor("linw_in", [F, OUT_DIM], fp32, kind="ExternalInput").ap()
    linb_in = nc.dram_tensor("linb_in", [128, OUT_DIM], fp32, kind="ExternalInput").ap()
    gidx_in = nc.dram_tensor("gidx_in", [128, NItot // 16], i16, kind="ExternalInput").ap()
    rgidx_in = nc.dram_tensor("rgidx_in", [128, 4 * NPCP // 16], i16, kind="ExternalInput").ap()
    p_in = nc.dram_tensor("p_in", [128, T], fp32, kind="ExternalInput").ap()
    out_t = nc.dram_tensor("out", [128, T, OUT_DIM], fp32, kind="ExternalOutput").ap()

    tshard = nc.dram_tensor("tshard", [NPCP, ROW], fp32)
    agout = nc.dram_tensor("agout", [NTAB, ROW], fp32, addr_space="Shared")
    table = agout
    subout = [nc.dram_tensor(f"subout{k}", [NPCP, ROW], fp32) for k in range(4)]

    with tile.TileContext(nc) as tc:
        from contextlib import ExitStack
        with ExitStack() as ctx:
            singles = ctx.enter_context(tc.tile_pool(name="singles", bufs=1))
            # --- constants ---
            w_sb = singles.tile([128, F + 2 * HEADS], fp32)
            nc.sync.dma_start(out=w_sb[:], in_=w_in[:])
            wd_sb = singles.tile([128, HEADS], fp32)
            nc.sync.dma_start(out=wd_sb[:], in_=wd_in[:])
            bias_sb = singles.tile([128, F], fp32)
            nc.sync.dma_start(out=bias_sb[:], in_=bias_in[:])
            linw_sb = singles.tile([F, OUT_DIM], fp32)
            nc.sync.dma_start(out=linw_sb[:], in_=linw_in[:])
            linb_sb = singles.tile([128, OUT_DIM], fp32)
            nc.sync.dma_start(out=linb_sb[:], in_=linb_in[:])
            p_sb = singles.tile([128, T], fp32)
            nc.sync.dma_start(out=p_sb[:], in_=p_in[:])
            rgidx_sb = singles.tile([128, 4 * NPCP // 16], i16)
            nc.sync.dma_start(out=rgidx_sb[:], in_=rgidx_in[:])
            ident = singles.tile([128, 128], fp32)
            make_identity(nc, ident[:])

            adst_nat = singles.tile([128, T, HEADS], fp32)
            # tstag rows [h(60) | a_src(4)] persist for phase-2 self-loops
            tstag = singles.tile([128, T, ROW], fp32)

            # ---------------- phase 0: table build ----------------
            NSLAB = 7
            SLAB = NPCP // NSLAB  # 1792 cols (14 tiles) per x slab
            with (
                tc.tile_pool(name="p0x", bufs=2) as p0x,
                tc.tile_pool(name="p0ps", bufs=4, space="PSUM") as p0ps,
            ):
                for s in range(NSLAB):
                    xsl = p0x.tile([128, SLAB], fp32, tag="xsl")
                    nc.sync.dma_start(out=xsl[:],
                                      in_=xT[:, SLAB * s:SLAB * (s + 1)])
                    for tt in range(SLAB // 128):
                        t = s * (SLAB // 128) + tt
                        hps = p0ps.tile([128, F + 2 * HEADS], fp32,
                                        space="PSUM", tag="hps")
                        nc.tensor.matmul(out=hps[:],
                                         lhsT=xsl[:, 128 * tt:128 * (tt + 1)],
                                         rhs=w_sb[:], start=True, stop=True)
                        nc.vector.tensor_copy(out=tstag[:, t, :],
                                              in_=hps[:, 0:ROW])
                        nc.vector.tensor_copy(out=adst_nat[:, t, :],
                                              in_=hps[:, ROW:ROW + HEADS])
                nc.sync.dma_start(
                    out=tshard[:].rearrange("(p t) d -> p (t d)", p=128),
                    in_=tstag[:].rearrange("p t d -> p (t d)"))
                nc.gpsimd.collective_compute(
                    "AllGather", AL.bypass,
                    replica_groups=[list(range(NCORES))],
                    ins=[tshard[0:SH, :]],
                    outs=[agout[:]],
                )

            # ---------------- phase 1: per-chunk CSR pipelines ----------------
            gcol_off = 0
            DMAXG = max(int(D[k].max()) for k in range(4))
            NIW = max(NI[k] // 16 for k in range(4))
            S_acc = singles.tile([128, T, ROW], fp32)
            with (
                tc.tile_pool(name="p1g", bufs=2) as p1g,
                tc.tile_pool(name="p1rg", bufs=1) as p1rg,
                tc.tile_pool(name="p1s", bufs=4) as p1s,
                tc.tile_pool(name="p1prod", bufs=1) as p1prod,
                tc.tile_pool(name="p1stag", bufs=1) as p1stag,
                tc.tile_pool(name="p1a", bufs=2) as p1a,
                tc.tile_pool(name="p1ps", bufs=2, space="PSUM") as p1ps,
                tc.tile_pool(name="p1idx", bufs=2) as p1idx,
            ):
                for k in range(4):
                    Dk = D[k]
                    gidx_k = p1idx.tile([128, NIW], i16, tag="gidx")
                    nc.sync.dma_start(
                        out=gidx_k[:, 0:NI[k] // 16],
                        in_=gidx_in[:, gcol_off:gcol_off + NI[k] // 16])
                    gcol_off += NI[k] // 16
                    kcol = 0
                    sstag = p1stag.tile([128, T, ROW], fp32, tag="sstag")
                    nc.vector.memset(sstag[:], 0.0)
                    # a_dst per sub position via matmul with wd (x chunk-sorted)
                    adst_sub = p1a.tile([128, T, HEADS], fp32, tag="adst_sub")
                    NSLAB = 7
                    SLAB = NPCP // NSLAB
                    for s in range(NSLAB):
                        xsl = p1a.tile([128, SLAB], fp32, tag="xs")
                        nc.sync.dma_start(out=xsl[:],
                                          in_=xs_in[k][:, SLAB * s:SLAB * (s + 1)])
                        for tt in range(SLAB // 128):
                            tau = s * (SLAB // 128) + tt
                            aps = p1ps.tile([128, HEADS], fp32, space="PSUM",
                                            tag="aps")
                            nc.tensor.matmul(
                                out=aps[:],
                                lhsT=xsl[:, 128 * tt:128 * (tt + 1)],
                                rhs=wd_sb[:], start=True, stop=True)
                            nc.vector.tensor_copy(out=adst_sub[:, tau, :],
                                                  in_=aps[:])
                    for group in _groups_of(Dk):
                        g_cols = int(sum(Dk[tau] for tau in group))
                        n_idx = 128 * g_cols
                        gt = p1g.tile([128, GROUP_COLS, ROW], fp32, tag="gt")
                        nc.gpsimd.dma_gather(
                            out_ap=gt[:, 0:g_cols, :],
                            in_ap=table[:][CHROWS * k:, :],
                            idxs_ap=gidx_k[:, kcol:kcol + n_idx // 16],
                            num_idxs=n_idx, num_idxs_reg=n_idx, elem_size=ROW,
                            single_packet=False)
                        kcol += n_idx // 16
                        o = 0
                        for (tau0, nt, Dv) in _runs_of(Dk, group):
                            nd = nt * Dv
                            gv = gt[:, o:o + nd, :]
                            o += nd
                            sv = p1s.tile([128, RUN_COLS, HEADS], fp32, tag="sv")
                            adst_b = bass.AP(
                                tensor=adst_sub.tensor,
                                offset=adst_sub[:, tau0, :].offset,
                                ap=[adst_sub.ap[0], [HEADS, nt], [0, Dv],
                                    [1, HEADS]])
                            nc.vector.tensor_tensor(out=sv[:, 0:nd, :],
                                                    in0=gv[:, :, F:F + HEADS],
                                                    in1=adst_b, op=AL.add)
                            ev = p1s.tile([128, RUN_COLS, HEADS], fp32, tag="ev")
                            nc.vector.tensor_scalar_mul(ev[:, 0:nd, :], sv[:, 0:nd, :], NEG_SLOPE)
                            nc.vector.tensor_tensor(out=ev[:, 0:nd, :], in0=sv[:, 0:nd, :],
                                                    in1=ev[:, 0:nd, :], op=AL.max)
                            wv = p1s.tile([128, RUN_COLS, HEADS], fp32, tag="wv")
                            nc.scalar.activation(out=wv[:, 0:nd, :], in_=ev[:, 0:nd, :],
                                                 func=AF.Exp)
                            wt = bass.AP(tensor=wv.tensor, offset=wv.offset,
                                         ap=[wv.ap[0], [HEADS * Dv, nt],
                                             [1, HEADS], [HEADS, Dv]])
                            nc.vector.tensor_reduce(
                                out=sstag[:, tau0:tau0 + nt, F:F + HEADS],
                                in_=wt, axis=mybir.AxisListType.X, op=AL.add)
                            prod = p1prod.tile([128, RUN_COLS, F], fp32, tag="prod")
                            w_b = bass.AP(tensor=wv.tensor, offset=wv.offset,
                                          ap=[wv.ap[0], [HEADS, nd], [1, HEADS], [0, HID]])
                            nc.vector.tensor_tensor(out=prod[:, 0:nd, :],
                                                    in0=gv[:, :, 0:F],
                                                    in1=w_b, op=AL.mult)
                            pt = bass.AP(tensor=prod.tensor, offset=prod.offset,
                                         ap=[prod.ap[0], [F * Dv, nt],
                                             [1, F], [F, Dv]])
                            nc.vector.tensor_reduce(
                                out=sstag[:, tau0:tau0 + nt, 0:F],
                                in_=pt, axis=mybir.AxisListType.X, op=AL.add)
                    nc.sync.dma_start(
                        out=subout[k][:].rearrange("(p t) d -> p (t d)", p=128),
                        in_=sstag[:].rearrange("p t d -> p (t d)"))
                    # regather this chunk's partials to natural order now so
                    # only the last one sits in the tail
                    rg = p1rg.tile([128, T, ROW], fp32, tag="rg")
                    nc.gpsimd.dma_gather(
                        out_ap=rg[:], in_ap=subout[k][:],
                        idxs_ap=rgidx_sb[:, k * (NPCP // 16):(k + 1) * (NPCP // 16)],
                        num_idxs=NPCP, num_idxs_reg=NPCP, elem_size=ROW,
                        single_packet=False)
                    if k == 0:
                        nc.vector.tensor_copy(out=S_acc[:], in_=rg[:])
                    else:
                        nc.vector.tensor_tensor(out=S_acc[:], in0=S_acc[:],
                                                in1=rg[:], op=AL.add)

            # ---------------- phase 2: combine ----------------
            with (
                tc.tile_pool(name="p2acc", bufs=1) as p2acc,
                tc.tile_pool(name="p2ps", bufs=2, space="PSUM") as p2ps,
                tc.tile_pool(name="p2t", bufs=4) as p2t,
            ):
                S = S_acc
                # self-loop contribution (natural order):
                #   wl = exp(leaky(asrc_nat + adst_nat));
                #   S.num += h_nat * wl ; S.den += wl
                ls = p2acc.tile([128, T, HEADS], fp32)
                wl = p2acc.tile([128, T, HEADS], fp32)
                nc.vector.tensor_tensor(out=ls[:], in0=tstag[:, :, F:F + HEADS],
                                        in1=adst_nat[:], op=AL.add)
                nc.vector.tensor_scalar_mul(wl[:], ls[:], NEG_SLOPE)
                nc.vector.tensor_tensor(out=wl[:], in0=ls[:], in1=wl[:], op=AL.max)
                nc.scalar.activation(out=wl[:], in_=wl[:], func=AF.Exp)
                nc.vector.tensor_tensor(out=S[:, :, F:F + HEADS],
                                        in0=S[:, :, F:F + HEADS], in1=wl[:],
                                        op=AL.add)
                wl_b = bass.AP(tensor=wl.tensor, offset=wl.offset,
                               ap=[wl.ap[0], [HEADS, T], [1, HEADS], [0, HID]])
                lnum = p2acc.tile([128, T, F], fp32)
                nc.vector.tensor_tensor(out=lnum[:], in0=tstag[:, :, 0:F],
                                        in1=wl_b, op=AL.mult)
                nc.vector.tensor_tensor(out=S[:, :, 0:F], in0=S[:, :, 0:F],
                                        in1=lnum[:], op=AL.add)
                # pad correction: pads are zero rows, each contributes
                # exp(leaky(a_dst)) to the denominator
                nc.vector.tensor_scalar_mul(ls[:], adst_nat[:], NEG_SLOPE)
                nc.vector.tensor_tensor(out=ls[:], in0=adst_nat[:], in1=ls[:], op=AL.max)
                nc.scalar.activation(out=ls[:], in_=ls[:], func=AF.Exp)
                p_b = bass.AP(tensor=p_sb.tensor, offset=p_sb.offset,
                              ap=[p_sb.ap[0], [1, T], [0, HEADS]])
                nc.vector.tensor_tensor(out=ls[:], in0=ls[:], in1=p_b, op=AL.mult)
                nc.vector.tensor_tensor(out=S[:, :, F:F + HEADS],
                                        in0=S[:, :, F:F + HEADS], in1=ls[:],
                                        op=AL.subtract)
                nc.vector.tensor_scalar_max(S[:, :, F:F + HEADS],
                                            S[:, :, F:F + HEADS], 1e-30)
                rec = p2acc.tile([128, T, HEADS], fp32)
                nc.vector.reciprocal(out=rec[:], in_=S[:, :, F:F + HEADS])
                y = p2acc.tile([128, T, F], fp32)
                rec_b = bass.AP(tensor=rec.tensor, offset=rec.offset,
                                ap=[rec.ap[0], [HEADS, T], [1, HEADS], [0, HID]])
                nc.vector.tensor_tensor(out=y[:], in0=S[:, :, 0:F], in1=rec_b,
                                        op=AL.mult)
                bias_b = bass.AP(tensor=bias_sb.tensor, offset=bias_sb.offset,
                                 ap=[bias_sb.ap[0], [0, T], [1, F]])
                nc.vector.tensor_tensor(out=y[:], in0=y[:], in1=bias_b, op=AL.add)
                # elu(y) = max(y,0) + exp(min(y,0)) - 1
                mn = p2acc.tile([128, T, F], fp32)
                nc.vector.tensor_scalar_min(mn[:], y[:], 0.0)
                nc.scalar.activation(out=mn[:], in_=mn[:], func=AF.Exp)
                nc.vector.tensor_scalar_max(y[:], y[:], 0.0)
                nc.vector.tensor_scalar_add(y[:], y[:], -1.0)
                nc.vector.tensor_tensor(out=y[:], in0=y[:], in1=mn[:], op=AL.add)
                zt_all = p2acc.tile([128, T, OUT_DIM], fp32)
                for t in range(T):
                    zps = p2ps.tile([F, 128], fp32, space="PSUM", tag="zps")
                    nc.tensor.transpose(out=zps[:], in_=y[:, t, :], identity=ident[:])
                    zT = p2t.tile([F, 128], fp32, tag="zT")
                    nc.vector.tensor_copy(out=zT[:], in_=zps[:])
                    ops_ = p2ps.tile([128, OUT_DIM], fp32, space="PSUM", tag="ops")
                    nc.tensor.matmul(out=ops_[:], lhsT=zT[:], rhs=linw_sb[:],
                                     start=True, stop=True)
                    nc.vector.tensor_copy(out=zt_all[:, t, :], in_=ops_[:])
                linb_b = bass.AP(tensor=linb_sb.tensor, offset=linb_sb.offset,
                                 ap=[linb_sb.ap[0], [0, T], [1, OUT_DIM]])
                nc.vector.tensor_tensor(out=zt_all[:], in0=zt_all[:],
                                        in1=linb_b, op=AL.add)
                mx = p2acc.tile([128, T, 1], fp32)
                nc.vector.tensor_reduce(out=mx[:], in_=zt_all[:],
                                        axis=mybir.AxisListType.X, op=AL.max)
                mx_b = bass.AP(tensor=mx.tensor, offset=mx.offset,
                               ap=[mx.ap[0], [1, T], [0, OUT_DIM]])
                nc.vector.tensor_tensor(out=zt_all[:], in0=zt_all[:],
                                        in1=mx_b, op=AL.subtract)
                qe = p2acc.tile([128, T, OUT_DIM], fp32)
                nc.scalar.activation(out=qe[:], in_=zt_all[:], func=AF.Exp)
                ssum = p2acc.tile([128, T, 1], fp32)
                nc.vector.tensor_reduce(out=ssum[:], in_=qe[:],
                                        axis=mybir.AxisListType.X, op=AL.add)
                nc.scalar.activation(out=ssum[:], in_=ssum[:], func=AF.Ln)
                ssum_b = bass.AP(tensor=ssum.tensor, offset=ssum.offset,
                                 ap=[ssum.ap[0], [1, T], [0, OUT_DIM]])
                nc.vector.tensor_tensor(out=zt_all[:], in0=zt_all[:],
                                        in1=ssum_b, op=AL.subtract)
                nc.sync.dma_start(out=out_t[:], in_=zt_all[:])

    nc.compile()
    return nc


_PROGRAM_CACHE = {}


LAST_EXEC_NS = None
LAST_TRACE = None


def kernel(**inputs):
    import os
    from concourse.bass_utils import run_bass_kernel_spmd

    x = np.asarray(inputs["x"], dtype=np.float32)
    ei = np.asarray(inputs["edge_index"])
    W = np.asarray(inputs["W"], dtype=np.float32)
    att_src = np.asarray(inputs["att_src"], dtype=np.float32)
    att_dst = np.asarray(inputs["att_dst"], dtype=np.float32)
    bias = np.asarray(inputs["bias"], dtype=np.float32)
    lin_w = np.asarray(inputs["lin_w"], dtype=np.float32)
    lin_b = np.asarray(inputs["lin_b"], dtype=np.float32)

    src = ei[0].astype(np.int64)
    dst = ei[1].astype(np.int64)

    D, per_core = _preprocess(src, dst)

    key = tuple(int(v) for k in range(4) for v in D[k])
    if key not in _PROGRAM_CACHE:
        _PROGRAM_CACHE[key] = _build_program(D)
    nc = _PROGRAM_CACHE[key]

    # shared parameter arrays; fold ws/wd into the phase-0 weight matrix
    wd_arr = np.einsum('ihc,hc->ih', W.reshape(IN_DIM, HEADS, HID),
                       att_dst).astype(np.float32)
    ws_arr = np.einsum('ihc,hc->ih', W.reshape(IN_DIM, HEADS, HID),
                       att_src).astype(np.float32)
    w_arr = np.concatenate([W.reshape(IN_DIM, F), ws_arr, wd_arr],
                           axis=1).astype(np.float32)
    bias_arr = np.tile(bias.reshape(1, F), (128, 1)).astype(np.float32)
    linb_arr = np.tile(lin_b.reshape(1, OUT_DIM), (128, 1)).astype(np.float32)

    # natural slot order: device slot (p, t) (col 128t+p of xT) holds
    # local node 98p+t
    p_grid, t_grid = np.meshgrid(np.arange(128), np.arange(T), indexing="ij")
    sigma = (p_grid * T + t_grid)  # [128, T] local id for slot (p, t)
    loc_nat = sigma.T.reshape(-1)  # column j=128t+p -> local id

    in_maps = []
    for c in range(NCORES):
        gidx, rgidx, Parr = _build_core_arrays(D, per_core[c])
        xs = np.zeros((NPCP, IN_DIM), np.float32)
        valid = loc_nat < NPC
        xs[valid] = x[c * NPC + loc_nat[valid]]
        im = {
            "xT": np.ascontiguousarray(xs.T),
            "w_in": w_arr,
            "wd_in": wd_arr,
            "bias_in": bias_arr,
            "linw_in": lin_w,
            "linb_in": linb_arr,
            "gidx_in": gidx,
            "rgidx_in": rgidx,
            "p_in": Parr,
        }
        # chunk-sorted x copies: col 128*tau + p' holds x of local node
        # order_k[128*tau + p']
        for k in range(4):
            order = per_core[c][k]["order"]
            loc_k = order[np.arange(NPCP)]  # rank -> local id
            # column j corresponds to rank j (tau = j//128? no: j = 128t+p
            # maps to rank 128t+p)
            xk = np.zeros((NPCP, IN_DIM), np.float32)
            vk = loc_k < NPC
            xk[vk] = x[c * NPC + loc_k[vk]]
            im[f"xs{k}"] = np.ascontiguousarray(xk.T)
        in_maps.append(im)

    trace = os.environ.get("KERNEL_TRACE") == "1"
    res = run_bass_kernel_spmd(nc, in_maps, list(range(NCORES)), trace=trace)
    global LAST_EXEC_NS, LAST_TRACE
    LAST_EXEC_NS = res.exec_time_ns
    LAST_TRACE = res.instructions_and_trace[1] if res.instructions_and_trace else None

    out = np.empty((N, OUT_DIM), np.float32)
    for c in range(NCORES):
        buf = res.results[c]["out"]  # [128, T, OUT_DIM], slot (p,t) = local 98p+t
        flat = buf.reshape(128 * T, OUT_DIM)
        out[c * NPC:(c + 1) * NPC] = flat[:NPC]
    return out


# revision 31
# speedup vs baseline: 1.0463x; 1.0463x over previous
"""GAT message-passing kernel for 8 Trainium2 NeuronCores.

Strategy (dst-sharded, per core):
  - Nodes are partitioned across 8 cores by destination id (12500 dst nodes
    per core); every real edge is owned by the core owning its destination.
    Self-loops are computed locally (no gather).
  - Phase 0: each core computes h = x @ W, a_src = <h, att_src>,
    a_dst = <h, att_dst> for its own nodes, packs [h(60) | a_src(4)] rows and
    AllGathers them into a full 100128-row table (each core contributes
    12516 rows: 12500 data + 16 zero rows used as pad targets).
  - Phase 1: per src-chunk k (4 chunks of 25032 table rows so chunk-local
    indices fit int16 for dma_gather), edges are laid out in a CSR slot grid
    [128 dst x D columns] per tile, degree-sorted per chunk so padding is
    small.  One dma_gather per group of tiles fetches packed rows per edge.
    a_dst in chunk-sorted order comes from host-permuted x copies via a tiny
    matmul against wd = W @ att_dst (folded on host), NOT via dma_gather.
    Unnormalized attention w = exp(leaky_relu(a_src + a_dst)) (softmax
    normalization cancels), then per-dst row-sums produce partial
    numerator [60] and denominator [4].
  - Phase 2: the 4 chunk-partial outputs are re-gathered into natural dst
    order, summed; self-loop contributions and pad corrections are applied
    analytically; then out = elu(num/den + bias) @ lin_w + lin_b
    -> log_softmax.
"""
import sys

sys.path.insert(0, "/opt/trn_rl_repo")

import numpy as np

N, E = 100000, 1600000
IN_DIM, HEADS, HID, OUT_DIM = 128, 4, 15, 10
NEG_SLOPE = 0.2
NCORES = 8
NPC = N // NCORES              # 12500 dst nodes per core
T = (NPC + 127) // 128         # 98 tiles
NPCP = 128 * T                 # 12544 padded locals
SH = NPC + 16                  # 12516 shard rows contributed to allgather
CHSZ = N // 4                  # 25000 nodes per chunk (2 cores)
CHROWS = 2 * SH                # 25032 table rows per chunk window
NTAB = NCORES * SH             # 100128 table rows
ROW = 64                       # packed row: h(60) + a_src(4)
F = HEADS * HID                # 60
GROUP_COLS = 96                # max CSR columns per dma_gather


def _wrap_idx(flat):
    """int32 flat index list -> [128, n//16] int16 wrapped layout."""
    n = len(flat)
    assert n % 16 == 0
    w = flat.astype(np.int16).reshape(n // 16, 16).T.copy()
    return np.tile(w, (8, 1))


def _preprocess(src, dst):
    """Build per-core schedules (no self-loops). Returns (D, per_core)."""
    core_of = dst // NPC
    per_core = []
    for c in range(NCORES):
        m = core_of == c
        s_c = src[m].astype(np.int64)
        d_loc = (dst[m] - c * NPC).astype(np.int64)
        chunks = []
        for k in range(4):
            km = (s_c // CHSZ) == k
            sk = s_c[km]
            # chunk-local table row: 2*SH*k + (sk - 25000k) + 16*(second core)
            loc = sk - k * CHSZ
            tloc = np.where(loc < NPC, loc, loc + 16).astype(np.int32)
            dk = d_loc[km].astype(np.int32)
            counts = np.bincount(dk, minlength=NPCP).astype(np.int32)
            order = np.argsort(-counts, kind="stable").astype(np.int32)
            oi = np.empty(NPCP, np.int32)
            oi[order] = np.arange(NPCP, dtype=np.int32)
            D_ck = counts[order[0::128]]
            chunks.append(dict(tloc=tloc, dk=dk, counts=counts, order=order,
                               oi=oi, D_ck=D_ck))
        per_core.append(chunks)
    D = [np.max([per_core[c][k]["D_ck"] for c in range(NCORES)], axis=0)
         .astype(np.int32) for k in range(4)]
    return D, per_core


def _build_core_arrays(D, chunks):
    """Per-core gather index arrays and pad counts."""
    gidx_parts = []
    rgidx_parts = []
    P = np.zeros(NPCP, np.float32)
    for k in range(4):
        d = chunks[k]
        Dk = D[k]
        base = np.concatenate([[0], np.cumsum(128 * Dk.astype(np.int64))])
        NI = int(base[-1])
        # pad target: zero row 12500 of the chunk's first core block
        idx_flat = np.full(NI, NPC, np.int32)
        o = np.argsort(d["dk"], kind="stable")
        dks = d["dk"][o]
        tls = d["tloc"][o]
        starts = np.concatenate([[0], np.cumsum(d["counts"])]).astype(np.int64)
        rank = np.arange(len(dks), dtype=np.int64) - starts[dks]
        oi = d["oi"][dks]
        tau, pp = oi // 128, oi % 128
        linear = base[tau] + rank * 128 + pp
        idx_flat[linear] = tls
        gidx_parts.append(_wrap_idx(idx_flat))
        # combine regather: final slot (p, t) i.e. local = 98p+t, output
        # linear i = t*128 + p  -> sub-out row = p'*T + tau
        loc = (np.arange(NPCP, dtype=np.int32).reshape(T, 128) % 128) * T \
            + np.arange(T, dtype=np.int32)[:, None]
        oiL = d["oi"][loc.reshape(-1)]
        subrow = (oiL % 128) * T + oiL // 128
        rgidx_parts.append(_wrap_idx(subrow.astype(np.int32)))
        P += (Dk[d["oi"] // 128] - d["counts"]).astype(np.float32)
    gidx = np.concatenate(gidx_parts, axis=1)
    rgidx = np.concatenate(rgidx_parts, axis=1)
    # P laid out [128, T] with [p, t] = P[98p + t]
    Parr = P[(np.arange(128)[:, None] * T + np.arange(T)[None, :])]
    return gidx, rgidx, Parr.astype(np.float32)


def _groups_of(Dk):
    """Split tiles into gather groups with <= GROUP_COLS columns."""
    groups = []
    cur = []
    cols = 0
    for tau in range(T):
        dcol = int(Dk[tau])
        if dcol == 0:
            continue
        if cols + dcol > GROUP_COLS and cur:
            groups.append(cur)
            cur = []
            cols = 0
        cur.append(tau)
        cols += dcol
    if cur:
        groups.append(cur)
    return groups


RUN_COLS = 48


def _runs_of(Dk, group):
    """Split a group's consecutive taus into equal-D runs <= RUN_COLS cols.

    Returns [(tau0, ntile, Dv), ...]; D is non-increasing so equal-D tiles
    are consecutive.
    """
    runs = []
    i = 0
    while i < len(group):
        Dv = int(Dk[group[i]])
        j = i
        cols = 0
        while j < len(group) and int(Dk[group[j]]) == Dv \
                and cols + Dv <= RUN_COLS:
            cols += Dv
            j += 1
        if j == i:
            j = i + 1
        runs.append((group[i], j - i, Dv))
        i = j
    return runs


def _build_program(D):
    import concourse.bass as bass
    import concourse.bacc as bacc
    import concourse.tile as tile
    from concourse import mybir
    from concourse.masks import make_identity

    fp32 = mybir.dt.float32
    i16 = mybir.dt.int16
    AL = mybir.AluOpType
    AF = mybir.ActivationFunctionType

    NI = [int((128 * D[k].astype(np.int64)).sum()) for k in range(4)]
    NItot = sum(NI)

    nc = bacc.Bacc("TRN2", target_bir_lowering=False, debug=False,
                   num_devices=NCORES)

    xT = nc.dram_tensor("xT", [128, NPCP], fp32, kind="ExternalInput").ap()
    # x columns permuted into each chunk's degree-sorted slot order
    xs_in = [nc.dram_tensor(f"xs{k}", [128, NPCP], fp32,
                            kind="ExternalInput").ap() for k in range(4)]
    # w68 = [W(60) | W@att_src(4) | W@att_dst(4)] folded on host
    w_in = nc.dram_tensor("w_in", [128, F + 2 * HEADS], fp32,
                          kind="ExternalInput").ap()
    wd_in = nc.dram_tensor("wd_in", [128, HEADS], fp32, kind="ExternalInput").ap()
    bias_in = nc.dram_tensor("bias_in", [128, F], fp32, kind="ExternalInput").ap()
    linw_in = nc.dram_tensor("linw_in", [F, OUT_DIM], fp32, kind="ExternalInput").ap()
    linb_in = nc.dram_tensor("linb_in", [128, OUT_DIM], fp32, kind="ExternalInput").ap()
    gidx_in = nc.dram_tensor("gidx_in", [128, NItot // 16], i16, kind="ExternalInput").ap()
    rgidx_in = nc.dram_tensor("rgidx_in", [128, 4 * NPCP // 16], i16, kind="ExternalInput").ap()
    p_in = nc.dram_tensor("p_in", [128, T], fp32, kind="ExternalInput").ap()
    out_t = nc.dram_tensor("out", [128, T, OUT_DIM], fp32, kind="ExternalOutput").ap()

    tshard = nc.dram_tensor("tshard", [NPCP, ROW], fp32)
    agout = nc.dram_tensor("agout", [NTAB, ROW], fp32, addr_space="Shared")
    table = agout
    subout = [nc.dram_tensor(f"subout{k}", [NPCP, ROW], fp32) for k in range(4)]

    with tile.TileContext(nc) as tc:
        from contextlib import ExitStack
        with ExitStack() as ctx:
            singles = ctx.enter_context(tc.tile_pool(name="singles", bufs=1))
            # --- constants ---
            w_sb = singles.tile([128, F + 2 * HEADS], fp32)
            nc.sync.dma_start(out=w_sb[:], in_=w_in[:])
            wd_sb = singles.tile([128, HEADS], fp32)
            nc.sync.dma_start(out=wd_sb[:], in_=wd_in[:])
            bias_sb = singles.tile([128, F], fp32)
            nc.sync.dma_start(out=bias_sb[:], in_=bias_in[:])
            linw_sb = singles.tile([F, OUT_DIM], fp32)
            nc.sync.dma_start(out=linw_sb[:], in_=linw_in[:])
            linb_sb = singles.tile([128, OUT_DIM], fp32)
            nc.sync.dma_start(out=linb_sb[:], in_=linb_in[:])
            p_sb = singles.tile([128, T], fp32)
            nc.sync.dma_start(out=p_sb[:], in_=p_in[:])
            rgidx_sb = singles.tile([128, 4 * NPCP // 16], i16)
            nc.sync.dma_start(out=rgidx_sb[:], in_=rgidx_in[:])
            ident = singles.tile([128, 128], fp32)
            make_identity(nc, ident[:])

            adst_nat = singles.tile([128, T, HEADS], fp32)
            # tstag rows [h(60) | a_src(4)] persist for phase-2 self-loops
            tstag = singles.tile([128, T, ROW], fp32)

            # ---------------- phase 0: table build ----------------
            NSLAB = 7
            SLAB = NPCP // NSLAB  # 1792 cols (14 tiles) per x slab
            with (
                tc.tile_pool(name="p0x", bufs=2) as p0x,
                tc.tile_pool(name="p0ps", bufs=4, space="PSUM") as p0ps,
            ):
                for s in range(NSLAB):
                    xsl = p0x.tile([128, SLAB], fp32, tag="xsl")
                    nc.sync.dma_start(out=xsl[:],
                                      in_=xT[:, SLAB * s:SLAB * (s + 1)])
                    for tt in range(SLAB // 128):
                        t = s * (SLAB // 128) + tt
                        hps = p0ps.tile([128, F + 2 * HEADS], fp32,
                                        space="PSUM", tag="hps")
                        nc.tensor.matmul(out=hps[:],
                                         lhsT=xsl[:, 128 * tt:128 * (tt + 1)],
                                         rhs=w_sb[:], start=True, stop=True)
                        nc.vector.tensor_copy(out=tstag[:, t, :],
                                              in_=hps[:, 0:ROW])
                        nc.vector.tensor_copy(out=adst_nat[:, t, :],
                                              in_=hps[:, ROW:ROW + HEADS])
                nc.sync.dma_start(
                    out=tshard[:].rearrange("(p t) d -> p (t d)", p=128),
                    in_=tstag[:].rearrange("p t d -> p (t d)"))
                nc.gpsimd.collective_compute(
                    "AllGather", AL.bypass,
                    replica_groups=[list(range(NCORES))],
                    ins=[tshard[0:SH, :]],
                    outs=[agout[:]],
                )

            # ---------------- phase 1: per-chunk CSR pipelines ----------------
            gcol_off = 0
            DMAXG = max(int(D[k].max()) for k in range(4))
            NIW = max(NI[k] // 16 for k in range(4))
            with (
                tc.tile_pool(name="p1g", bufs=3) as p1g,
                tc.tile_pool(name="p1s", bufs=4) as p1s,
                tc.tile_pool(name="p1prod", bufs=2) as p1prod,
                tc.tile_pool(name="p1stag", bufs=1) as p1stag,
                tc.tile_pool(name="p1a", bufs=2) as p1a,
                tc.tile_pool(name="p1ps", bufs=2, space="PSUM") as p1ps,
                tc.tile_pool(name="p1idx", bufs=2) as p1idx,
            ):
                for k in range(4):
                    Dk = D[k]
                    gidx_k = p1idx.tile([128, NIW], i16, tag="gidx")
                    nc.sync.dma_start(
                        out=gidx_k[:, 0:NI[k] // 16],
                        in_=gidx_in[:, gcol_off:gcol_off + NI[k] // 16])
                    gcol_off += NI[k] // 16
                    kcol = 0
                    sstag = p1stag.tile([128, T, ROW], fp32, tag="sstag")
                    nc.vector.memset(sstag[:], 0.0)
                    # a_dst per sub position via matmul with wd (x chunk-sorted)
                    adst_sub = p1a.tile([128, T, HEADS], fp32, tag="adst_sub")
                    NSLAB = 7
                    SLAB = NPCP // NSLAB
                    for s in range(NSLAB):
                        xsl = p1a.tile([128, SLAB], fp32, tag="xs")
                        nc.sync.dma_start(out=xsl[:],
                                          in_=xs_in[k][:, SLAB * s:SLAB * (s + 1)])
                        for tt in range(SLAB // 128):
                            tau = s * (SLAB // 128) + tt
                            aps = p1ps.tile([128, HEADS], fp32, space="PSUM",
                                            tag="aps")
                            nc.tensor.matmul(
                                out=aps[:],
                                lhsT=xsl[:, 128 * tt:128 * (tt + 1)],
                                rhs=wd_sb[:], start=True, stop=True)
                            nc.vector.tensor_copy(out=adst_sub[:, tau, :],
                                                  in_=aps[:])
                    for group in _groups_of(Dk):
                        g_cols = int(sum(Dk[tau] for tau in group))
                        n_idx = 128 * g_cols
                        gt = p1g.tile([128, GROUP_COLS, ROW], fp32, tag="gt")
                        nc.gpsimd.dma_gather(
                            out_ap=gt[:, 0:g_cols, :],
                            in_ap=table[:][CHROWS * k:, :],
                            idxs_ap=gidx_k[:, kcol:kcol + n_idx // 16],
                            num_idxs=n_idx, num_idxs_reg=n_idx, elem_size=ROW,
                            single_packet=False)
                        kcol += n_idx // 16
                        o = 0
                        for (tau0, nt, Dv) in _runs_of(Dk, group):
                            nd = nt * Dv
                            gv = gt[:, o:o + nd, :]
                            o += nd
                            sv = p1s.tile([128, RUN_COLS, HEADS], fp32, tag="sv")
                            adst_b = bass.AP(
                                tensor=adst_sub.tensor,
                                offset=adst_sub[:, tau0, :].offset,
                                ap=[adst_sub.ap[0], [HEADS, nt], [0, Dv],
                                    [1, HEADS]])
                            nc.vector.tensor_tensor(out=sv[:, 0:nd, :],
                                                    in0=gv[:, :, F:F + HEADS],
                                                    in1=adst_b, op=AL.add)
                            ev = p1s.tile([128, RUN_COLS, HEADS], fp32, tag="ev")
                            nc.vector.tensor_scalar_mul(ev[:, 0:nd, :], sv[:, 0:nd, :], NEG_SLOPE)
                            nc.vector.tensor_tensor(out=ev[:, 0:nd, :], in0=sv[:, 0:nd, :],
                                                    in1=ev[:, 0:nd, :], op=AL.max)
                            wv = p1s.tile([128, RUN_COLS, HEADS], fp32, tag="wv")
                            nc.scalar.activation(out=wv[:, 0:nd, :], in_=ev[:, 0:nd, :],
                                                 func=AF.Exp)
                            wt = bass.AP(tensor=wv.tensor, offset=wv.offset,
                                         ap=[wv.ap[0], [HEADS * Dv, nt],
                                             [1, HEADS], [HEADS, Dv]])
                            nc.vector.tensor_reduce(
                                out=sstag[:, tau0:tau0 + nt, F:F + HEADS],
                                in_=wt, axis=mybir.AxisListType.X, op=AL.add)
                            prod = p1prod.tile([128, RUN_COLS, F], fp32, tag="prod")
                            w_b = bass.AP(tensor=wv.tensor, offset=wv.offset,
                                          ap=[wv.ap[0], [HEADS, nd], [1, HEADS], [0, HID]])
                            nc.vector.tensor_tensor(out=prod[:, 0:nd, :],
                                                    in0=gv[:, :, 0:F],
                                                    in1=w_b, op=AL.mult)
                            pt = bass.AP(tensor=prod.tensor, offset=prod.offset,
                                         ap=[prod.ap[0], [F * Dv, nt],
                                             [1, F], [F, Dv]])
                            nc.vector.tensor_reduce(
                                out=sstag[:, tau0:tau0 + nt, 0:F],
                                in_=pt, axis=mybir.AxisListType.X, op=AL.add)
                    nc.sync.dma_start(
                        out=subout[k][:].rearrange("(p t) d -> p (t d)", p=128),
                        in_=sstag[:].rearrange("p t d -> p (t d)"))

            # ---------------- phase 2: combine ----------------
            with (
                tc.tile_pool(name="p2s", bufs=2) as p2s,
                tc.tile_pool(name="p2acc", bufs=1) as p2acc,
                tc.tile_pool(name="p2ps", bufs=2, space="PSUM") as p2ps,
                tc.tile_pool(name="p2t", bufs=4) as p2t,
            ):
                S = p2acc.tile([128, T, ROW], fp32)
                for k in range(4):
                    rg = p2s.tile([128, T, ROW], fp32, tag="rg")
                    nc.gpsimd.dma_gather(
                        out_ap=rg[:], in_ap=subout[k][:],
                        idxs_ap=rgidx_sb[:, k * (NPCP // 16):(k + 1) * (NPCP // 16)],
                        num_idxs=NPCP, num_idxs_reg=NPCP, elem_size=ROW, single_packet=False)
                    if k == 0:
                        nc.vector.tensor_copy(out=S[:], in_=rg[:])
                    else:
                        nc.vector.tensor_tensor(out=S[:], in0=S[:], in1=rg[:], op=AL.add)
                # self-loop contribution (natural order):
                #   wl = exp(leaky(asrc_nat + adst_nat));
                #   S.num += h_nat * wl ; S.den += wl
                ls = p2acc.tile([128, T, HEADS], fp32)
                wl = p2acc.tile([128, T, HEADS], fp32)
                nc.vector.tensor_tensor(out=ls[:], in0=tstag[:, :, F:F + HEADS],
                                        in1=adst_nat[:], op=AL.add)
                nc.vector.tensor_scalar_mul(wl[:], ls[:], NEG_SLOPE)
                nc.vector.tensor_tensor(out=wl[:], in0=ls[:], in1=wl[:], op=AL.max)
                nc.scalar.activation(out=wl[:], in_=wl[:], func=AF.Exp)
                nc.vector.tensor_tensor(out=S[:, :, F:F + HEADS],
                                        in0=S[:, :, F:F + HEADS], in1=wl[:],
                                        op=AL.add)
                wl_b = bass.AP(tensor=wl.tensor, offset=wl.offset,
                               ap=[wl.ap[0], [HEADS, T], [1, HEADS], [0, HID]])
                lnum = p2acc.tile([128, T, F], fp32)
                nc.vector.tensor_tensor(out=lnum[:], in0=tstag[:, :, 0:F],
                                        in1=wl_b, op=AL.mult)
                nc.vector.tensor_tensor(out=S[:, :, 0:F], in0=S[:, :, 0:F],
                                        in1=lnum[:], op=AL.add)
                # pad correction: pads are zero rows, each contributes
                # exp(leaky(a_dst)) to the denominator
                nc.vector.tensor_scalar_mul(ls[:], adst_nat[:], NEG_SLOPE)
                nc.vector.tensor_tensor(out=ls[:], in0=adst_nat[:], in1=ls[:], op=AL.max)
                nc.scalar.activation(out=ls[:], in_=ls[:], func=AF.Exp)
                p_b = bass.AP(tensor=p_sb.tensor, offset=p_sb.offset,
                              ap=[p_sb.ap[0], [1, T], [0, HEADS]])
                nc.vector.tensor_tensor(out=ls[:], in0=ls[:], in1=p_b, op=AL.mult)
                nc.vector.tensor_tensor(out=S[:, :, F:F + HEADS],
                                        in0=S[:, :, F:F + HEADS], in1=ls[:],
                                        op=AL.subtract)
                nc.vector.tensor_scalar_max(S[:, :, F:F + HEADS],
                                            S[:, :, F:F + HEADS], 1e-30)
                rec = p2acc.tile([128, T, HEADS], fp32)
                nc.vector.reciprocal(out=rec[:], in_=S[:, :, F:F + HEADS])
                y = p2acc.tile([128, T, F], fp32)
                rec_b = bass.AP(tensor=rec.tensor, offset=rec.offset,
                                ap=[rec.ap[0], [HEADS, T], [1, HEADS], [0, HID]])
                nc.vector.tensor_tensor(out=y[:], in0=S[:, :, 0:F], in1=rec_b,
                                        op=AL.mult)
                bias_b = bass.AP(tensor=bias_sb.tensor, offset=bias_sb.offset,
                                 ap=[bias_sb.ap[0], [0, T], [1, F]])
                nc.vector.tensor_tensor(out=y[:], in0=y[:], in1=bias_b, op=AL.add)
                # elu(y) = max(y,0) + exp(min(y,0)) - 1
                mn = p2acc.tile([128, T, F], fp32)
                nc.vector.tensor_scalar_min(mn[:], y[:], 0.0)
                nc.scalar.activation(out=mn[:], in_=mn[:], func=AF.Exp)
                nc.vector.tensor_scalar_max(y[:], y[:], 0.0)
                nc.vector.tensor_scalar_add(y[:], y[:], -1.0)
                nc.vector.tensor_tensor(out=y[:], in0=y[:], in1=mn[:], op=AL.add)
                zt_all = p2acc.tile([128, T, OUT_DIM], fp32)
                for t in range(T):
                    zps = p2ps.tile([F, 128], fp32, space="PSUM", tag="zps")
                    nc.tensor.transpose(out=zps[:], in_=y[:, t, :], identity=ident[:])
                    zT = p2t.tile([F, 128], fp32, tag="zT")
                    nc.vector.tensor_copy(out=zT[:], in_=zps[:])
                    ops_ = p2ps.tile([128, OUT_DIM], fp32, space="PSUM", tag="ops")
                    nc.tensor.matmul(out=ops_[:], lhsT=zT[:], rhs=linw_sb[:],
                                     start=True, stop=True)
                    nc.vector.tensor_copy(out=zt_all[:, t, :], in_=ops_[:])
                linb_b = bass.AP(tensor=linb_sb.tensor, offset=linb_sb.offset,
                                 ap=[linb_sb.ap[0], [0, T], [1, OUT_DIM]])
                nc.vector.tensor_tensor(out=zt_all[:], in0=zt_all[:],
                                        in1=linb_b, op=AL.add)
                mx = p2acc.tile([128, T, 1], fp32)
                nc.vector.tensor_reduce(out=mx[:], in_=zt_all[:],
                                        axis=mybir.AxisListType.X, op=AL.max)
                mx_b = bass.AP(tensor=mx.tensor, offset=mx.offset,
                               ap=[mx.ap[0], [1, T], [0, OUT_DIM]])
                nc.vector.tensor_tensor(out=zt_all[:], in0=zt_all[:],
                                        in1=mx_b, op=AL.subtract)
                qe = p2acc.tile([128, T, OUT_DIM], fp32)
                nc.scalar.activation(out=qe[:], in_=zt_all[:], func=AF.Exp)
                ssum = p2acc.tile([128, T, 1], fp32)
                nc.vector.tensor_reduce(out=ssum[:], in_=qe[:],
                                        axis=mybir.AxisListType.X, op=AL.add)
                nc.scalar.activation(out=ssum[:], in_=ssum[:], func=AF.Ln)
                ssum_b = bass.AP(tensor=ssum.tensor, offset=ssum.offset,
                                 ap=[ssum.ap[0], [1, T], [0, OUT_DIM]])
                nc.vector.tensor_tensor(out=zt_all[:], in0=zt_all[:],
                                        in1=ssum_b, op=AL.subtract)
                nc.sync.dma_start(out=out_t[:], in_=zt_all[:])

    nc.compile()
    return nc


_PROGRAM_CACHE = {}


LAST_EXEC_NS = None
LAST_TRACE = None


def kernel(**inputs):
    import os
    from concourse.bass_utils import run_bass_kernel_spmd

    x = np.asarray(inputs["x"], dtype=np.float32)
    ei = np.asarray(inputs["edge_index"])
    W = np.asarray(inputs["W"], dtype=np.float32)
    att_src = np.asarray(inputs["att_src"], dtype=np.float32)
    att_dst = np.asarray(inputs["att_dst"], dtype=np.float32)
    bias = np.asarray(inputs["bias"], dtype=np.float32)
    lin_w = np.asarray(inputs["lin_w"], dtype=np.float32)
    lin_b = np.asarray(inputs["lin_b"], dtype=np.float32)

    src = ei[0].astype(np.int64)
    dst = ei[1].astype(np.int64)

    D, per_core = _preprocess(src, dst)

    key = tuple(int(v) for k in range(4) for v in D[k])
    if key not in _PROGRAM_CACHE:
        _PROGRAM_CACHE[key] = _build_program(D)
    nc = _PROGRAM_CACHE[key]

    # shared parameter arrays; fold ws/wd into the phase-0 weight matrix
    wd_arr = np.einsum('ihc,hc->ih', W.reshape(IN_DIM, HEADS, HID),
                       att_dst).astype(np.float32)
    ws_arr = np.einsum('ihc,hc->ih', W.reshape(IN_DIM, HEADS, HID),
                       att_src).astype(np.float32)
    w_arr = np.concatenate([W.reshape(IN_DIM, F), ws_arr, wd_arr],
                           axis=1).astype(np.float32)
    bias_arr = np.tile(bias.reshape(1, F), (128, 1)).astype(np.float32)
    linb_arr = np.tile(lin_b.reshape(1, OUT_DIM), (128, 1)).astype(np.float32)

    # natural slot order: device slot (p, t) (col 128t+p of xT) holds
    # local node 98p+t
    p_grid, t_grid = np.meshgrid(np.arange(128), np.arange(T), indexing="ij")
    sigma = (p_grid * T + t_grid)  # [128, T] local id for slot (p, t)
    loc_nat = sigma.T.reshape(-1)  # column j=128t+p -> local id

    in_maps = []
    for c in range(NCORES):
        gidx, rgidx, Parr = _build_core_arrays(D, per_core[c])
        xs = np.zeros((NPCP, IN_DIM), np.float32)
        valid = loc_nat < NPC
        xs[valid] = x[c * NPC + loc_nat[valid]]
        im = {
            "xT": np.ascontiguousarray(xs.T),
            "w_in": w_arr,
            "wd_in": wd_arr,
            "bias_in": bias_arr,
            "linw_in": lin_w,
            "linb_in": linb_arr,
            "gidx_in": gidx,
            "rgidx_in": rgidx,
            "p_in": Parr,
        }
        # chunk-sorted x copies: col 128*tau + p' holds x of local node
        # order_k[128*tau + p']
        for k in range(4):
            order = per_core[c][k]["order"]
            loc_k = order[np.arange(NPCP)]  # rank -> local id
            # column j corresponds to rank j (tau = j//128? no: j = 128t+p
            # maps to rank 128t+p)
            xk = np.zeros((NPCP, IN_DIM), np.float32)
            vk = loc_k < NPC
            xk[vk] = x[c * NPC + loc_k[vk]]
            im[f"xs{k}"] = np.ascontiguousarray(xk.T)
        in_maps.append(im)

    trace = os.environ.get("KERNEL_TRACE") == "1"
    res = run_bass_kernel_spmd(nc, in_maps, list(range(NCORES)), trace=trace)
    global LAST_EXEC_NS, LAST_TRACE
    LAST_EXEC_NS = res.exec_time_ns
    LAST_TRACE = res.instructions_and_trace[1] if res.instructions_and_trace else None

    out = np.empty((N, OUT_DIM), np.float32)
    for c in range(NCORES):
        buf = res.results[c]["out"]  # [128, T, OUT_DIM], slot (p,t) = local 98p+t
        flat = buf.reshape(128 * T, OUT_DIM)
        out[c * NPC:(c + 1) * NPC] = flat[:NPC]
    return out
